# revision 3
# baseline (speedup 1.0000x reference)
"""CenterLoss kernel for Trainium2 (Bass/Tile), 8-core data-parallel.

loss = sum_i ||x_i - centers[labels_i]||^2
  x: (65536, 512) f32, labels: (65536,) int, centers: (512, 512) f32

Per-core plan (8192 rows each), using the expansion
  loss = sum x^2 - 2*sum_{c,d} S[c,d]*centers[c,d] + sum_c count_c*||C_c||^2
with S = onehot(labels)^T @ x computed on the PE via one-hot DoubleRow fp8
matmuls. The third term (counts * ||C||^2) is computed on the host from the
int labels + centers (no x involved), so the device only computes
  r1 = sum x^2        (ACT Square accum, from the exact f32 stream)
  r2 = -2*sum S.*C    (DVE STT over PSUM S against the SBUF centers)
and reduces r1+r2 across partitions with a ones-vector matmul so the
output is a single [1,1] scalar per core (one DMA descriptor, short tail).

All x traffic is HWDGE on the sync ring (SWDGE casting DMA is Q7
descriptor-gen bound at ~190 GB/s read-side and made chunk data late in the
baseline; HWDGE sustains ~420 GB/s). Small inputs (iota/labf) ride the sync
ring FIRST so one-hot building can start by ~8us; centers ride the scalar
ring (only needed at the tail). f32->fp8 casts are split between DVE
tensor_copy (2x mode) and ACT activation-Copy to balance the two engines;
sum(x^2) runs on ACT from the f32 tiles (exact, independent of the casts).
A few warmup matmuls on a memset tile get the PE HAM un-throttled before
the first real group.
"""

import sys

import numpy as np

sys.path.insert(0, "/opt/trn_rl_repo")

N_CORES = 8
B = 65536
D = 512
B_L = B // N_CORES  # 8192 rows per core

# x chunks in compute order: (rows, ring). One HWDGE ring tops out at
# ~320 GB/s descriptor-gen, so x is striped across BOTH HWDGE rings
# (sync + scalar) and the mid chunks ride the gpsimd SWDGE casting DMA
# with FLAT access patterns (one descriptor per partition instead of one
# per 2KB row) so Q7 descriptor generation keeps up. SWDGE chunks land
# directly as fp8 (no on-chip cast; their sumsq reads fp8, error ~2e-4).
CHUNKS = [
    (256, "sync"),
    (768, "scalar"),
    (1024, "sync"),
    (2048, "gpsimd"),
    (2048, "gpsimd"),
    (1024, "scalar"),
    (768, "sync"),
    (256, "scalar"),
]
CHUNK_ROWS = [r for r, _ in CHUNKS]
assert sum(CHUNK_ROWS) == B_L
assert all((r // 128) % 2 == 0 for r in CHUNK_ROWS)
NCH = D // 128  # 4 class chunks
N_CHUNKS = len(CHUNK_ROWS)

# which HWDGE chunks get their f32->fp8 cast on DVE (rest on ACT)
DVE_CAST_CHUNKS = {1, 5}

N_WARMUP_MM = 8  # junk matmuls to lift the PE HAM throttle before real work

_CACHE = {}


def _build():
    """Trace the Bass/Tile program once; returns the compiled Bacc module."""
    if "nc" in _CACHE:
        return _CACHE["nc"]

    import concourse.bacc as bacc
    import concourse.mybir as mybir
    import concourse.tile as tile

    f32 = mybir.dt.float32
    fp8 = mybir.dt.float8e4

    nc = bacc.Bacc("TRN2", debug=False, num_devices=N_CORES)
    x_t = nc.dram_tensor("x", [B_L, D], f32, kind="ExternalInput")
    iota_t = nc.dram_tensor("iota16", [128, D], mybir.dt.float16, kind="ExternalInput")
    labf_t = nc.dram_tensor("labf", [128, B_L // 128], f32, kind="ExternalInput")
    c_t = nc.dram_tensor("centers", [D, D], f32, kind="ExternalInput")
    out_t = nc.dram_tensor("out", [1, 1], f32, kind="ExternalOutput")

    with tile.TileContext(nc) as tc:
        with (
            tc.tile_pool(name="misc", bufs=1) as misc_pool,
            tc.tile_pool(name="psum", bufs=1, space="PSUM") as psum_pool,
        ):
            # small inputs FIRST on the sync HWDGE ring (finish ~1us after
            # stream start, ahead of the x chunks queued behind them)
            iota_sb = misc_pool.tile([128, D], mybir.dt.float16)
            nc.sync.dma_start(iota_sb[:], iota_t.ap())
            labf_sb = misc_pool.tile([128, B_L // 128], f32)
            nc.sync.dma_start(labf_sb[:], labf_t.ap())
            # centers on the scalar HWDGE ring (needed only by the r2 tail)
            cent_sb = misc_pool.tile([128, NCH, D], f32)
            nc.scalar.dma_start(
                cent_sb[:], c_t.ap().rearrange("(n p) d -> p n d", p=128)
            )

            acc_x2 = misc_pool.tile([128, N_CHUNKS], f32)
            r2acc = misc_pool.tile([128, NCH], f32)
            junk_dve = misc_pool.tile([128, 1], f32)
            junk_act = misc_pool.tile([128, 1], f32)
            r1 = misc_pool.tile([128, 1], f32)
            r2 = misc_pool.tile([128, 1], f32)
            total = misc_pool.tile([128, 1], f32)
            ones_sb = misc_pool.tile([128, 1], f32)
            res_sb = misc_pool.tile([128, 1], f32)
            warm8 = misc_pool.tile([128, 2, D], fp8)

            S_all = psum_pool.tile([128, NCH, D], f32, name="S_all")
            S_ps = [S_all[:, c, :] for c in range(NCH)]
            warm_ps = psum_pool.tile([128, D], f32, name="warm_ps")
            red_ps = psum_pool.tile([128, 1], f32, name="red_ps")

            # memsets (DVE): warmup operand + the ones vector for the final
            # cross-partition reduce
            nc.vector.memset(warm8[:], 0.0)
            nc.vector.memset(ones_sb[:], 1.0)

            # PE warmup: junk DoubleRow matmuls on the memset tile so the
            # HAM clock-gate opens (~3.4us of busy) before the first real
            # group; results go to a scratch PSUM bank
            for _ in range(N_WARMUP_MM):
                nc.tensor.matmul(
                    warm_ps[:],
                    lhsT=warm8[:, :, 0:128],
                    rhs=warm8[:],
                    start=True,
                    stop=True,
                    perf_mode=mybir.MatmulPerfMode.DoubleRow,
                )

            # static x tiles per chunk: f32 landing pad + fp8 cast output
            x32 = [
                None
                if ring == "gpsimd"
                else misc_pool.tile([128, r // 128, D], f32, name=f"x32_{i}")
                for i, (r, ring) in enumerate(CHUNKS)
            ]
            x8 = [
                misc_pool.tile([128, r // 128, D], fp8, name=f"x8_{i}")
                for i, r in enumerate(CHUNK_ROWS)
            ]

            x_ap = x_t.ap()
            qcs = [r // 128 for r in CHUNK_ROWS]
            toff = [sum(qcs[:i]) for i in range(N_CHUNKS)]  # labf col offset
            goff = [sum(q // 2 for q in qcs[:i]) for i in range(N_CHUNKS)]
            n_groups = B_L // 256  # 32 DoubleRow matmul groups

            # all x chunk DMAs trigger upfront, striped across the two
            # HWDGE rings + SWDGE (contiguous per-partition layout:
            # partition p holds rows [lo+p*qc, ...)). SWDGE casting DMAs
            # use FLAT [128, qc*512] APs so Q7 emits one big descriptor
            # per partition rather than one per 2KB row.
            lo = 0
            for ci, (rows, ring) in enumerate(CHUNKS):
                if ring == "gpsimd":
                    srcf = x_ap[lo : lo + rows, :].rearrange(
                        "(p q) d -> p (q d)", p=128
                    )
                    dstf = x8[ci][:].rearrange("p q d -> p (q d)")
                    nc.gpsimd.dma_start(dstf, srcf)
                else:
                    src = x_ap[lo : lo + rows, :].rearrange(
                        "(p q) d -> p q d", p=128
                    )
                    getattr(nc, ring).dma_start(x32[ci][:], src)
                lo += rows

            for ci, (rows, ring) in enumerate(CHUNKS):
                qc = qcs[ci]
                # f32 -> fp8 cast per 512-row slab (SWDGE chunks arrive
                # pre-cast), on the assigned engine
                if ring != "gpsimd":
                    n_sl = (qc + 3) // 4
                    for k in range(n_sl):
                        sl = slice(4 * k, min(4 * k + 4, qc))
                        if ci in DVE_CAST_CHUNKS:
                            nc.vector.tensor_copy(
                                x8[ci][:, sl, :], x32[ci][:, sl, :]
                            )
                        else:
                            nc.scalar.activation(
                                x8[ci][:, sl, :],
                                x32[ci][:, sl, :],
                                mybir.ActivationFunctionType.Copy,
                            )
                # sum(x^2) for the chunk on ACT (exact f32 where present,
                # fp8 for the SWDGE pre-cast chunks)
                x_src = x8[ci] if ring == "gpsimd" else x32[ci]
                x_flat = x_src[:].rearrange("p q d -> p (q d)")
                nc.scalar.activation(
                    junk_act[:].broadcast_to(x_flat.shape),
                    x_flat,
                    mybir.ActivationFunctionType.Square,
                    accum_out=acc_x2[:, ci : ci + 1],
                )
                # per 256-row group: one-hot build (DVE) + 4 DoubleRow matmuls
                for j in range(qc // 2):
                    oh = misc_pool.tile([128, 2, D], fp8, tag="oh", bufs=16)
                    for u in range(2):
                        tcol = toff[ci] + 2 * j + u
                        nc.vector.tensor_scalar(
                            out=oh[:, u, :],
                            in0=iota_sb[:],
                            scalar1=labf_sb[:, tcol : tcol + 1],
                            scalar2=None,
                            op0=mybir.AluOpType.is_equal,
                        )
                    g = goff[ci] + j
                    first = g == 0
                    last = g == n_groups - 1
                    for c in range(NCH):
                        nc.tensor.matmul(
                            S_ps[c],
                            lhsT=oh[:, :, c * 128 : (c + 1) * 128],
                            rhs=x8[ci][:, 2 * j : 2 * j + 2, :],
                            start=first,
                            stop=last,
                            perf_mode=mybir.MatmulPerfMode.DoubleRow,
                        )

            # tail: r2_c = -2*sum_d S[c,d]*C[c,d] per class chunk (each STT
            # can start as soon as that chunk's last matmul retires)
            for c in range(NCH):
                nc.vector.scalar_tensor_tensor(
                    out=junk_dve[:].broadcast_to(S_ps[c].shape),
                    in0=S_ps[c],
                    scalar=-2.0,
                    in1=cent_sb[:, c, :],
                    op0=mybir.AluOpType.mult,
                    op1=mybir.AluOpType.mult,
                    accum_out=r2acc[:, c : c + 1],
                )
            nc.vector.tensor_reduce(
                r1[:], acc_x2[:], axis=mybir.AxisListType.X, op=mybir.AluOpType.add
            )
            nc.vector.tensor_reduce(
                r2[:], r2acc[:], axis=mybir.AxisListType.X, op=mybir.AluOpType.add
            )
            nc.vector.tensor_tensor(
                total[:], r1[:], r2[:], op=mybir.AluOpType.add
            )
            # cross-partition reduce on the PE: [1,1] = total^T @ ones
            nc.tensor.matmul(
                red_ps[0:1, 0:1],
                lhsT=total[:],
                rhs=ones_sb[:],
                start=True,
                stop=True,
            )
            nc.vector.tensor_copy(res_sb[0:1, 0:1], red_ps[0:1, 0:1])
            nc.sync.dma_start(out_t.ap(), res_sb[0:1, 0:1])

    nc.compile()
    _CACHE["nc"] = nc
    return nc


def _prep_inputs(x, labels, centers):
    """Shard full inputs into the 8 per-core input maps."""
    x = np.asarray(x, dtype=np.float32)
    labels = np.asarray(labels)
    centers = np.ascontiguousarray(np.asarray(centers, dtype=np.float32))
    iota16 = np.ascontiguousarray(np.tile(np.arange(D, dtype=np.float16), (128, 1)))
    in_maps = []
    for cidx in range(N_CORES):
        xs = np.ascontiguousarray(x[cidx * B_L : (cidx + 1) * B_L])
        lab = np.asarray(labels[cidx * B_L : (cidx + 1) * B_L], dtype=np.int64)
        # labf[p, t]: label of the row that lands at (partition p, q-col t),
        # chunk ci contributing qc = rows/128 q-cols, row = lo + p*qc + qq
        cols = []
        lo = 0
        for rows in CHUNK_ROWS:
            qc = rows // 128
            cols.append(lab[lo : lo + rows].reshape(128, qc))
            lo += rows
        labf = np.ascontiguousarray(np.concatenate(cols, axis=1).astype(np.float32))
        in_maps.append(
            {
                "x": xs,
                "iota16": iota16,
                "labf": labf,
                "centers": centers,
            }
        )
    return in_maps


def _run(x, labels, centers, trace=False):
    from concourse import bass_utils

    nc = _build()
    in_maps = _prep_inputs(x, labels, centers)
    res = bass_utils.run_bass_kernel_spmd(
        nc, in_maps, core_ids=list(range(N_CORES)), trace=trace
    )
    total = np.float64(0.0)
    for r in res.results:
        total += np.sum(r["out"].astype(np.float64))
    # r3 = sum_c count_c * ||C_c||^2 from the labels histogram (host-side;
    # needs only labels+centers, no x)
    lab = np.asarray(labels).astype(np.int64)
    bc = np.bincount(lab, minlength=D).astype(np.float64)
    csq = np.einsum(
        "cd,cd->c",
        np.asarray(centers, dtype=np.float64),
        np.asarray(centers, dtype=np.float64),
    )
    total += float(np.dot(bc, csq))
    return np.array(total, dtype=np.float32), res


def kernel(x, labels, centers):
    out, _ = _run(x, labels, centers, trace=False)
    return out


def kernel_traced(x, labels, centers):
    return _run(x, labels, centers, trace=True)
